# revision 55
# baseline (speedup 1.0000x reference)
"""EEND-SS loss device kernel (raw Bass, SPMD over 8 cores).

Device computes, per core (B_LOC=4 samples):
  - "gram":  Gram matrix of [sep rows(12) | src rows(12) | ones] over T,
             chunk-blocked so host extracts all pairwise dots / sums / sq-sums.
  - "dgram": Gram of [logp | log1mp] x [tgt | 1-tgt] over T_sub=1000 for the
             diarization BCE (labels nearest-neighbor subsampled on the fly
             via strided DVE copy).
Host does the tiny O(B) permutation-invariant (PIT) math + existence BCE.

Memory-bound regime. Per-core HBM traffic = 12.7MB. Measured HW facts
this schedule is built around:
  - SDMA packet cost ~9ns + bytes/27.5GB/s per engine, 16 engines, and
    the engine buffer splits packets >1024B; a DMA descriptor covers one
    partition's contiguous T-run, so packet size = 4*W bytes. W=250
    (1000B packets, ~352GB/s) is the sweet spot -> 4 phases of 250.
  - dma_start costs a FIXED ~0.7us regardless of descriptor count, and
    the HWDGE ring holds ~2048 descriptors with whole-DMA enqueue, so
    use few big (384-desc) DMAs; ring-full parks self-pace the issuer.
  - the HW activity manager permits a ~25-30us full-rate burst, then
    duty-cycles DMA to ~half rate for 13-17us -> front-load everything.
  - the two queues get ~equal SDMA service regardless of queued bytes,
    so both rings must carry equal data (2 samples each + half the
    labels each).
  - compute is sub-chunked at 65 columns: repack (DVE rows 0:17 at
    ~1.4 el/ns, ACT rows 17:24 at ~0.6 el/ns, f32->bf16 CAST into the
    blocked layout) and the PE matmuls trail each phase's data by one
    chunk, so the post-DMA tail is repack(250 cols)/2-engines + one
    chunk of matmuls + psum copy + output DMA.

  DMA (row-major)  ->  staging[ph]  [128, 24*W] f32
  repack chunks    ->  blocked[ph]  [128, 26*W] bf16
  PE per 5-chunk block: psum[125,125] += lhsT.T @ lhsT

Only SP (sync) and Activation (scalar) have HWDGE rings: sync carries
samples 0-1 + diar + labels(s0,s1) + outputs; scalar carries samples
2-3 + labels(s2,s3) and issues phase ph+2 right after its phase-ph
repack (never before: a ring-full park there would starve the repack).

Blocked layout: free = (g, r, c): index = 130*g + 5*r + c, r in 0..25
 (rows 0..23 = data row r = s*6 + t*3 + i, row 24 = ones, row 25 = pad),
 c in 0..4, chunk = 5*g + c.  Partition p holds T positions
 [TCOLS*p, TCOLS*(p+1)), chunk n is column n of that view.
 Host: Gamma[ra, rb] = sum_c psum[5*ra+c, 5*rb+c]
"""

import numpy as np
from contextlib import ExitStack

import concourse.bass as bass
from concourse import mybir

F32 = mybir.dt.float32
BF16 = mybir.dt.bfloat16
AFT = mybir.ActivationFunctionType

C = 3
B_LOC = 4
P = 128
PD = 125           # diar partition count
TSUB = 1000
KSUB = TSUB // PD  # 8 t_sub positions per partition
NROW = 26          # 24 data rows + ones(24) + pad(25)
NDATA = 24
BLK = 5 * NROW     # 130: free elems per chunk-group block
CW = 65            # compute chunk width (columns, multiple of 5)


def build_nc(T=128000, WS=(250, 250, 250, 250), TF_STRIDE=8):
    """Build the per-core Bass program. Returns (nc, meta)."""
    TCOLS = T // P
    assert TCOLS * P == T
    WS = list(WS)
    NPH = len(WS)
    # staging/blocked slots are 1:1 with phases; total SBUF is constant
    # (sum(WS) == TCOLS) regardless of NPH
    assert sum(WS) == TCOLS
    for w in WS:
        assert w % 5 == 0
    OFF = [sum(WS[:i]) for i in range(NPH)]
    TF = TSUB * TF_STRIDE

    # compute chunks: per phase, column ranges of <= CW cols (5-aligned)
    CHUNKS = []   # list of (ph, c0, c1)
    for ph, w in enumerate(WS):
        c0 = 0
        while c0 < w:
            c1 = min(c0 + CW, w)
            CHUNKS.append((ph, c0, c1))
            c0 = c1
    NCH = [sum(1 for ph, _, _ in CHUNKS if ph == p) for p in range(NPH)]
    CUM = [sum(NCH[:p]) for p in range(NPH)]

    nc = bass.Bass(trn_type="TRN2", target_bir_lowering=False, debug=False,
                   enable_partition_id=False)

    sep = nc.dram_tensor("sep", [B_LOC, C, T], F32, kind="ExternalInput").ap()
    src = nc.dram_tensor("src", [B_LOC, C, T], F32, kind="ExternalInput").ap()
    diar = nc.dram_tensor("diar", [B_LOC, TSUB, C], F32, kind="ExternalInput").ap()
    lab = nc.dram_tensor("lab", [B_LOC, TF, C], F32, kind="ExternalInput").ap()

    gram_out = nc.dram_tensor("gram", [PD, PD + 27], F32, kind="ExternalOutput").ap()

    # SBUF (slot i belongs to phase i exclusively)
    stg = [nc.alloc_sbuf_tensor(f"stg{i}", [P, NDATA * WS[i]], F32).ap()
           for i in range(NPH)]
    blk = [nc.alloc_sbuf_tensor(f"blk{i}", [P, NROW * WS[i]], BF16).ap()
           for i in range(NPH)]
    p0 = nc.alloc_sbuf_tensor("p0", [PD, B_LOC * KSUB * C], F32).ap()   # (s,k,j)
    lf = nc.alloc_sbuf_tensor("lf", [PD, B_LOC * (TF // PD) * C], F32).ap()  # full labels
    # ll/rr layout: free = (k, q, s, j): k-slice contiguous 24 for matmul lhsT
    ll = nc.alloc_sbuf_tensor("ll", [PD, KSUB * 2 * B_LOC * C], F32).ap()
    rr = nc.alloc_sbuf_tensor("rr", [PD, KSUB * 2 * B_LOC * C], F32).ap()
    out_sb = nc.alloc_sbuf_tensor("out_sb", [PD, PD + 27], F32).ap()

    # PSUM
    ps_g = nc.alloc_psum_tensor("ps_g", [PD, PD], F32).ap()
    ps_d = nc.alloc_psum_tensor("ps_d", [NDATA, NDATA], F32).ap()

    # helper views
    def stg3(ph):  # [p, r, n]
        return stg[ph].rearrange("p (r n) -> p r n", r=NDATA)

    def stg4(ph):  # [p, r, g, c]
        return stg[ph].rearrange("p (r g c) -> p r g c", r=NDATA, c=5)

    def blk4(ph):  # [p, r, g, c] view of blocked (g, r, c) layout
        return blk[ph].rearrange("p (g r c) -> p r g c", r=NROW, c=5)

    # repack row split (chunked DVE CAST ~1.4 el/ns, ACT ~0.6 el/ns)
    ROWS = {"dve": (0, 17), "act": (17, 24)}

    with ExitStack() as ctx:
        st_sems = [ctx.enter_context(nc.semaphore(f"st_sem{i}")) for i in range(NPH)]
        pdma_sem = ctx.enter_context(nc.semaphore("pdma_sem"))
        ldma_sem = ctx.enter_context(nc.semaphore("ldma_sem"))
        odma_sem = ctx.enter_context(nc.semaphore("odma_sem"))
        rpk_dve = ctx.enter_context(nc.semaphore("rpk_dve"))   # counts chunks
        rpk_act = ctx.enter_context(nc.semaphore("rpk_act"))   # counts chunks
        rpk_gp = ctx.enter_context(nc.semaphore("rpk_gp"))
        act_sem = ctx.enter_context(nc.semaphore("act_sem"))
        dve_sem = ctx.enter_context(nc.semaphore("dve_sem"))
        pe_sem = ctx.enter_context(nc.semaphore("pe_sem"))
        block = ctx.enter_context(nc.Block())

        def issue_phase_dmas(eng, ph, samples):
            # one DMA per (sample, tensor): 384 descriptors / 384KB each.
            # dma_start costs a FIXED ~0.7us regardless of desc count, so big
            # DMAs keep the ring issue rate (~550GB/s) well above the wire;
            # parks on ring-full (capacity ~2048 descs) self-pace the issuer.
            s3 = stg3(ph)
            for s in samples:
                for t, big in enumerate((sep, src)):
                    r0 = s * 6 + t * 3
                    src_ap = big[s].rearrange("i (p n) -> p i n", p=P)[
                        :, :, OFF[ph]:OFF[ph] + WS[ph]]
                    eng.dma_start(out=s3[:, r0:r0 + 3, :], in_=src_ap
                                  ).then_inc(st_sems[ph], 16)

        @block.sync
        def _(sync: bass.BassEngine):
            # sync has no other duties, so parking in dma_start on queue-full
            # backpressure is harmless here
            issue_phase_dmas(sync, 0, (0, 1))
            if NPH > 1:
                issue_phase_dmas(sync, 1, (0, 1))
            sync.dma_start(
                out=p0.rearrange("p (s x) -> p s x", s=B_LOC),
                in_=diar.rearrange("s (p k) j -> p s (k j)", p=PD),
            ).then_inc(pdma_sem, 16)
            lfv = lf.rearrange("p (s x) -> p s x", s=B_LOC)
            lv = lab.rearrange("s (p e) j -> p s (e j)", p=PD)
            for s in (0, 1):
                sync.dma_start(out=lfv[:, s], in_=lv[:, s]).then_inc(ldma_sem, 16)
            for ph in range(2, NPH):
                issue_phase_dmas(sync, ph, (0, 1))

            # outputs: diar gram as soon as its psum copy lands, big gram
            # right after the final matmul's psum copy
            sync.wait_ge(dve_sem, 3)
            sync.dma_start(out=gram_out[0:NDATA, PD:PD + NDATA],
                           in_=out_sb[0:NDATA, PD:PD + NDATA]
                           ).then_inc(odma_sem, 16)
            sync.wait_ge(dve_sem, 4)
            sync.dma_start(out=gram_out[:, 0:PD], in_=out_sb[:, 0:PD]
                           ).then_inc(odma_sem, 16)
            sync.wait_ge(odma_sem, 32)

        @block.gpsimd
        def _(gpsimd: bass.BassEngine):
            # ones(row 24) + pad(row 25) once per blocked slot; no repack here
            # (gpsimd copies measured ~6x slower than DVE)
            for i in range(NPH):
                ap1 = blk[i].rearrange("p (g x) -> p g x", x=BLK)[
                    :, :, 5 * NDATA:5 * NROW]
                gpsimd.memset(ap1, 1.0).then_inc(rpk_gp, 1)

        @block.scalar
        def _(scalar: bass.BassEngine):
            issue_phase_dmas(scalar, 0, (2, 3))
            if NPH > 1:
                issue_phase_dmas(scalar, 1, (2, 3))
            r0, r1 = ROWS["act"]

            def diar_acts():
                rrk = rr.rearrange("p (k q s j) -> p k q s j", k=KSUB, q=2, s=B_LOC)
                llk = ll.rearrange("p (k q s j) -> p k q s j", k=KSUB, q=2, s=B_LOC)
                p0k = p0.rearrange("p (s k j) -> p k s j", s=B_LOC, k=KSUB)
                scalar.wait_ge(pdma_sem, 16)
                scalar.activation(llk[:, :, 0, :, :], p0k, AFT.Ln).then_inc(act_sem, 1)
                scalar.activation(llk[:, :, 1, :, :], p0k, AFT.Ln,
                                  scale=-1.0, bias=1.0).then_inc(act_sem, 1)
                scalar.wait_ge(dve_sem, 1)
                scalar.activation(rrk[:, :, 1, :, :], rrk[:, :, 0, :, :], AFT.Copy,
                                  scale=-1.0, bias=1.0).then_inc(act_sem, 1)

            lfv = lf.rearrange("p (s x) -> p s x", s=B_LOC)
            lv = lab.rearrange("s (p e) j -> p s (e j)", p=PD)
            for ph in range(NPH):
                scalar.wait_ge(st_sems[ph], 16 * 8)
                bv, sv = blk4(ph), stg4(ph)
                for (p_, c0, c1) in CHUNKS:
                    if p_ != ph:
                        continue
                    scalar.activation(
                        bv[:, r0:r1, c0 // 5:c1 // 5, :],
                        sv[:, r0:r1, c0 // 5:c1 // 5, :],
                        AFT.Copy).then_inc(rpk_act, 1)
                # issue-ahead AFTER the repack of ph: by now phase ph has
                # fully retired from the ring, so these enqueue park-free,
                # and a park can't delay this phase's repack
                if ph + 2 < NPH:
                    issue_phase_dmas(scalar, ph + 2, (2, 3))
                if ph == 0:
                    for s in (2, 3):
                        scalar.dma_start(out=lfv[:, s], in_=lv[:, s]
                                         ).then_inc(ldma_sem, 16)
                if ph == min(1, NPH - 1):
                    diar_acts()

        @block.vector
        def _(vector: bass.BassEngine):
            r0, r1 = ROWS["dve"]

            def diar_dve():
                # nearest-neighbor label subsample: lf (s, 8k+f, j), f=0
                rrk = rr.rearrange("p (k q s j) -> p k q s j", k=KSUB, q=2, s=B_LOC)
                lf5 = lf.rearrange("p (s k f j) -> p k s f j", s=B_LOC, k=KSUB,
                                   f=(TF // PD) // KSUB)[:, :, :, 0, :]
                vector.wait_ge(ldma_sem, 16 * B_LOC)
                vector.tensor_copy(rrk[:, :, 0, :, :], lf5).then_inc(dve_sem, 1)
                vector.wait_ge(act_sem, 2)
                vector.tensor_scalar_max(ll[:, :], ll[:, :], -100.0).then_inc(dve_sem, 1)

            for ph in range(NPH):
                vector.wait_ge(st_sems[ph], 16 * 8)
                bv, sv = blk4(ph), stg4(ph)
                for (p_, c0, c1) in CHUNKS:
                    if p_ != ph:
                        continue
                    vector.tensor_copy(
                        bv[:, r0:r1, c0 // 5:c1 // 5, :],
                        sv[:, r0:r1, c0 // 5:c1 // 5, :],
                    ).then_inc(rpk_dve, 1)
                if ph == min(1, NPH - 1):
                    diar_dve()

            vector.wait_ge(pe_sem, min(2, NPH - 1) + 2)
            vector.tensor_copy(out_sb[0:NDATA, PD:PD + NDATA], ps_d
                               ).then_inc(dve_sem, 1)
            vector.wait_ge(pe_sem, NPH + 1)
            vector.tensor_copy(out_sb[:, 0:PD], ps_g).then_inc(dve_sem, 1)

        @block.tensor
        def _(tensor: bass.BassEngine):
            nmm = 0
            total_mm = TCOLS // 5
            tensor.wait_ge(rpk_gp, NPH)
            for ph in range(NPH):
                b = blk[ph]
                for ci, (p_, c0, c1) in enumerate(CHUNKS):
                    if p_ != ph:
                        continue
                    tensor.wait_ge(rpk_dve, ci + 1)
                    tensor.wait_ge(rpk_act, ci + 1)
                    for g in range(c0 // 5, c1 // 5):
                        ap = b[:, BLK * g: BLK * g + 125]
                        mm = tensor.matmul(ps_g, ap, ap,
                                           start=(nmm == 0), stop=(nmm == total_mm - 1))
                        nmm += 1
                mm.then_inc(pe_sem, 1)
                if ph == min(2, NPH - 1):
                    # diar matmuls mid-stream; lhsT k-slices contiguous 24 cols
                    tensor.wait_ge(pdma_sem, 16)
                    tensor.wait_ge(ldma_sem, 16 * B_LOC)
                    tensor.wait_ge(act_sem, 3)
                    tensor.wait_ge(dve_sem, 2)
                    nd = 2 * B_LOC * C  # 24
                    for k in range(KSUB):
                        dmm = tensor.matmul(ps_d, ll[:, k * nd:(k + 1) * nd],
                                            rr[:, k * nd:(k + 1) * nd],
                                            start=(k == 0), stop=(k == KSUB - 1))
                    dmm.then_inc(pe_sem, 1)

    meta = dict(T=T, WS=WS)
    return nc, meta


# ---------------- host side ----------------

EPS = 1e-8
LAM_SISNR, LAM_DIAR, LAM_EXIST = 1.0, 0.2, 0.2
from itertools import permutations
PERMS = np.array(list(permutations(range(C))), dtype=np.int64)  # [6, 3]


def host_gamma_fp32(g125):
    """g125 [125,125] -> Gamma [25,25]; m = 5*r + c."""
    return np.einsum('acbc->ab', g125.reshape(25, 5, 25, 5).astype(np.float64))


def _clog(x):
    with np.errstate(divide='ignore'):
        return np.maximum(np.log(x), -100.0)


def host_finalize(gammas, dgrams, exist_probs, num_speakers, T=128000):
    """gammas: list of [25,25] float64 per core; dgrams list of [24,24].
    Returns the 5 scalars (np.float32)."""
    B = len(gammas) * B_LOC
    ns = np.asarray(num_speakers).astype(np.int64)

    S = np.zeros((B, C, C), np.float64)
    D = np.zeros((B, C, C), np.float64)
    for core, (gam, dg) in enumerate(zip(gammas, dgrams)):
        dg = dg.astype(np.float64)
        for s in range(B_LOC):
            b = core * B_LOC + s
            e_rows = [s * 6 + i for i in range(3)]
            t_rows = [s * 6 + 3 + j for j in range(3)]
            dot_raw = gam[np.ix_(e_rows, t_rows)]            # [i, j]
            sep_sq = np.array([gam[r, r] for r in e_rows])
            src_sq = np.array([gam[r, r] for r in t_rows])
            sum_sep = gam[e_rows, 24]
            sum_src = gam[t_rows, 24]

            dot = dot_raw - np.outer(sum_sep, sum_src) / T
            est_sq = sep_sq - sum_sep ** 2 / T               # [i]
            tgt_sq = src_sq - sum_src ** 2 / T               # [j]

            alpha = dot / (tgt_sq[None, :] + EPS)
            sig = alpha * alpha * tgt_sq[None, :] + EPS
            noise = est_sq[:, None] - 2.0 * alpha * dot + alpha * alpha * tgt_sq[None, :] + EPS
            S[b] = 10.0 * np.log10(sig / noise)

            A = dg[s * 3:s * 3 + 3, s * 3:s * 3 + 3]
            Bm = dg[12 + s * 3:12 + s * 3 + 3, 12 + s * 3:12 + s * 3 + 3]
            D[b] = -(A + Bm) / TSUB

    n_spk = np.clip(ns, 1, C)
    slot = np.arange(C)
    slot_mask = (slot[None, :] < n_spk[:, None]).astype(np.float64)
    valid = np.all((PERMS[None, :, :] < n_spk[:, None, None])
                   | (slot[None, None, :] >= n_spk[:, None, None]), axis=-1)

    S_perm = S[:, PERMS, slot]                               # [B, 6, 3]
    sisnr_mean = (S_perm * slot_mask[:, None, :]).sum(-1) / n_spk[:, None]
    sisnr_loss_p = np.where(valid, -sisnr_mean, np.inf)
    best = sisnr_loss_p.min(axis=-1)
    loss_sisnr = best.mean()
    mean_sisnr = (-best).mean()

    D_perm = D[:, PERMS, slot]
    diar_p = (D_perm * slot_mask[:, None, :]).sum(-1) / n_spk[:, None]
    loss_diar = np.where(valid, diar_p, np.inf).min(axis=-1).mean()

    ep = np.asarray(exist_probs, np.float64)
    n_ex = np.minimum(ns, C)
    ex_tgt = (np.arange(C + 1)[None, :] < n_ex[:, None]).astype(np.float64)
    bce_ex = -(ex_tgt * _clog(ep) + (1.0 - ex_tgt) * _clog(1.0 - ep))
    loss_exist = bce_ex.mean()

    total = LAM_SISNR * loss_sisnr + LAM_DIAR * loss_diar + LAM_EXIST * loss_exist
    return tuple(np.float32(v) for v in
                 (total, loss_sisnr, loss_diar, loss_exist, mean_sisnr))


def shard_inputs(separated, diar_probs, sources, labels, n_cores=8):
    maps = []
    for c in range(n_cores):
        sl = slice(B_LOC * c, B_LOC * (c + 1))
        maps.append({
            "sep": np.ascontiguousarray(separated[sl], dtype=np.float32),
            "src": np.ascontiguousarray(sources[sl], dtype=np.float32),
            "diar": np.ascontiguousarray(diar_probs[sl], dtype=np.float32),
            "lab": np.ascontiguousarray(labels[sl], dtype=np.float32),
        })
    return maps


# ---------------- kernel entry (self-contained) ----------------

N_CORES = 8
_CACHE = {}


def _get_nc():
    if "nc" not in _CACHE:
        _CACHE["nc"] = build_nc(T=128000, WS=(250, 250, 250, 250))[0]
    return _CACHE["nc"]


def kernel(separated, diar_probs, exist_probs, sources, labels, num_speakers):
    """EEND-SS loss on 8 NeuronCores: batch sharded 4 samples/core; device
    computes the big time-axis Grams; host does the tiny PIT/existence math."""
    from concourse.bass_utils import run_bass_kernel_spmd

    separated = np.asarray(separated)
    diar_probs = np.asarray(diar_probs)
    exist_probs = np.asarray(exist_probs)
    sources = np.asarray(sources)
    labels = np.asarray(labels)
    num_speakers = np.asarray(num_speakers)

    nc = _get_nc()
    in_maps = shard_inputs(separated, diar_probs, sources, labels, N_CORES)
    res = run_bass_kernel_spmd(nc, in_maps, list(range(N_CORES)))

    # gram output [125, 152]: cols 0:125 = chunk-blocked Gram, cols 125:149
    # rows 0:24 carry the diar Gram
    gammas = [host_gamma_fp32(res.results[c]["gram"][:, :PD]) for c in range(N_CORES)]
    dgrams = [res.results[c]["gram"][:NDATA, PD:PD + NDATA] for c in range(N_CORES)]
    return host_finalize(gammas, dgrams, exist_probs, num_speakers, T=128000)
